# revision 3
# baseline (speedup 1.0000x reference)
"""Trainium2 Bass kernel for BitFlipLinear: y[b,s,o] = sum_i x[b,s,i]*W[o,i] + bias[o].

Strategy
--------
Data-parallel over the batch dim: each of the 8 NeuronCores computes one
[4096,4096] @ [4096,4096]^T matmul (137 GFLOP/core).

Precision trick: W's values are {0,1,3} - exactly representable in bf16.
Split x = x_hi + x_lo (x_hi = bf16(x), x_lo = bf16(x - x_hi)) and run TWO
bf16 matmuls accumulated in fp32 PSUM.  Residual error ~2^-18 per element,
i.e. effectively fp32-accurate output at bf16 TensorE speed.

Per-core pipeline (single NEFF, Tile-scheduled):
  phase W : cast W fp32 -> bf16 in DRAM (SWDGE cast-DMA, DRAM->DRAM)
  phase X : per 128-row block, load x fp32, DVE-split into x_hi/x_lo bf16,
            write back to DRAM scratch
  matmul  : for each o-half (2048 cols): keep W^T[:, o_half] bf16 resident in
            SBUF (16.75 MB, loaded once via XBAR transpose-DMA); stream
            x_hi^T/x_lo^T 128-row tiles via transpose-DMA; accumulate
            4 PSUM banks per s-tile over all 32 k-tiles x {hi,lo}.
            The bias is pre-loaded into PSUM with a K=2 matmul
            (ones[2,128]^T @ [bias_hi;bias_lo][2,512]) as the start=True
            instruction of each accumulation group, so the epilogue is a
            plain PSUM->SBUF copy + DMA out.
"""

import os
import sys

for _p in ("/opt/trn_rl_repo",):
    if os.path.isdir(_p) and _p not in sys.path:
        sys.path.append(_p)

import numpy as np

B, S, K, O = 8, 4096, 4096, 4096
N_CORES = 8
O_CHUNK = 2048          # o columns resident per pass (4 PSUM banks of 512 fp32)
CONV_I = 512            # i-chunk for the x hi/lo split stage
W_CAST_ROWS = 1024      # rows per W fp32->bf16 cast DMA
BIAS_CH = 1024          # bias prep chunk

_NC_CACHE = {}


def build_nc(S=S, K=K, O=O, O_CHUNK=O_CHUNK, enable_asserts=False):
    """Build + compile the single-core Bass program (same NEFF runs on all cores)."""
    import concourse.bacc as bacc
    import concourse.tile as tile
    import concourse.mybir as mybir

    f32 = mybir.dt.float32
    bf16 = mybir.dt.bfloat16
    P = 128
    NB = O_CHUNK // 512       # PSUM banks per o-chunk
    NOH = O // O_CHUNK        # number of o-chunks
    ST = S // P               # s-tiles
    KO = K // P               # k-tiles (contraction)
    NCV = K // CONV_I         # conv chunks per s-tile

    assert S % P == 0 and K % P == 0 and O % O_CHUNK == 0 and O_CHUNK % 512 == 0

    nc = bacc.Bacc("TRN2", target_bir_lowering=False, debug=False,
                   enable_asserts=enable_asserts)

    ap_x = nc.dram_tensor("x", [S, K], f32, kind="ExternalInput").ap()
    ap_w = nc.dram_tensor("w", [O, K], f32, kind="ExternalInput").ap()
    ap_bias = nc.dram_tensor("bias", [O], f32, kind="ExternalInput").ap()
    ap_y = nc.dram_tensor("y", [S, O], f32, kind="ExternalOutput").ap()

    with tile.TileContext(nc) as tc:
        with (
            tc.tile_pool(name="dram", bufs=1, space="DRAM") as dram,
            tc.tile_pool(name="const", bufs=1) as const,
            tc.tile_pool(name="bstage", bufs=1) as bstage,
            tc.tile_pool(name="convin", bufs=2) as convin,
            tc.tile_pool(name="convout", bufs=2) as convout,
            tc.tile_pool(name="wres", bufs=1) as wresp,
            tc.tile_pool(name="xts", bufs=2) as xtsp,
            tc.tile_pool(name="outp", bufs=2) as outp,
            tc.tile_pool(name="psum", bufs=2, space="PSUM") as psum,
        ):
            wb = dram.tile([O, K], bf16)     # bf16 copy of W
            xhi = dram.tile([S, K], bf16)    # bf16 high part of x
            xlo = dram.tile([S, K], bf16)    # bf16 low (residual) part of x

            # ---- bias prep: bias2[0,:]=bf16(bias), bias2[1,:]=bf16(bias-hi) ----
            bias2 = const.tile([2, O], bf16)
            ones2 = const.tile([2, P], bf16)
            nc.vector.memset(ones2[:], 1.0)
            bch = min(BIAS_CH, O)
            for c in range(O // bch):
                sl = slice(c * bch, (c + 1) * bch)
                bst = bstage.tile([1, bch], f32)
                nc.scalar.dma_start(bst[:], ap_bias[None, sl])
                nc.vector.tensor_copy(bias2[0:1, sl], bst[:])
                # DVE can't write at base partition 1 -> stage lo on partition
                # 0 and move it with a small SBUF->SBUF DMA.
                blo = bstage.tile([1, bch], bf16, tag="blo")
                nc.vector.tensor_sub(blo[:], bst[:], bias2[0:1, sl])
                nc.scalar.dma_start(bias2[1:2, sl], blo[:])

            # ---- W fp32 -> bf16 (DRAM->DRAM cast on SWDGE) ----
            wch = min(W_CAST_ROWS, O)
            for c in range(O // wch):
                sl = slice(c * wch, (c + 1) * wch)
                nc.gpsimd.dma_start(wb[sl, :], ap_w[sl, :])

            for oh in range(NOH):
                # resident W^T slab: wres[pi, ko, o] = W[oh*O_CHUNK+o, ko*128+pi]
                wres = wresp.tile([P, KO, O_CHUNK], bf16)
                for c in range(O_CHUNK // 512):
                    o0 = oh * O_CHUNK + c * 512
                    nc.sync.dma_start(
                        wres[:, :, c * 512:(c + 1) * 512],
                        wb[o0:o0 + 512, :],
                        transpose=True,
                    )

                for st in range(ST):
                    rows = slice(st * P, (st + 1) * P)
                    if oh == 0:
                        # split x rows into hi/lo bf16, store to DRAM scratch
                        for c in range(NCV):
                            cols = slice(c * CONV_I, (c + 1) * CONV_I)
                            xin = convin.tile([P, CONV_I], f32)
                            nc.scalar.dma_start(xin[:], ap_x[rows, cols])
                            ch = convout.tile([P, 2, CONV_I], bf16)
                            nc.vector.tensor_copy(ch[:, 0], xin[:])
                            nc.vector.tensor_sub(ch[:, 1], xin[:], ch[:, 0])
                            nc.scalar.dma_start(xhi[rows, cols], ch[:, 0])
                            nc.scalar.dma_start(xlo[rows, cols], ch[:, 1])

                    # transposed x tiles: xt[pi, h, ko, s] = x_h[s, ko*128+pi]
                    xt = xtsp.tile([P, 2, KO, P], bf16)
                    nc.sync.dma_start(xt[:, 0], xhi[rows, :], transpose=True)
                    nc.sync.dma_start(xt[:, 1], xlo[rows, :], transpose=True)

                    pt = psum.tile([P, O_CHUNK], f32)
                    # bias seeds each accumulation group (start=True clears bank)
                    for ob in range(NB):
                        b0 = oh * O_CHUNK + ob * 512
                        nc.tensor.matmul(
                            pt[:, ob * 512:(ob + 1) * 512],
                            ones2[:], bias2[:, b0:b0 + 512],
                            start=True, stop=False,
                        )
                    for ko in range(KO):
                        for h in range(2):
                            last = (ko == KO - 1) and (h == 1)
                            lhsT = xt[:, h, ko, :]
                            for ob in range(NB):
                                nc.tensor.matmul(
                                    pt[:, ob * 512:(ob + 1) * 512],
                                    lhsT,
                                    wres[:, ko, ob * 512:(ob + 1) * 512],
                                    start=False, stop=last,
                                )

                    ot = outp.tile([P, O_CHUNK], f32)
                    nc.any.tensor_copy(ot[:], pt[:])
                    nc.scalar.dma_start(
                        ap_y[rows, oh * O_CHUNK:(oh + 1) * O_CHUNK], ot[:]
                    )

    nc.compile()
    return nc


def _get_nc():
    key = (S, K, O, O_CHUNK)
    if key not in _NC_CACHE:
        _NC_CACHE[key] = build_nc(S, K, O, O_CHUNK)
    return _NC_CACHE[key]


def make_in_maps(x, weight, bias):
    x = np.ascontiguousarray(np.asarray(x, dtype=np.float32))
    weight = np.ascontiguousarray(np.asarray(weight, dtype=np.float32))
    bias = np.ascontiguousarray(np.asarray(bias, dtype=np.float32))
    assert x.shape == (B, S, K), x.shape
    return [
        {"x": np.ascontiguousarray(x[b]), "w": weight, "bias": bias}
        for b in range(B)
    ]


def kernel(x, weight, bias):
    from concourse.bass_utils import run_bass_kernel_spmd

    nc = _get_nc()
    in_maps = make_in_maps(x, weight, bias)
    res = run_bass_kernel_spmd(nc, in_maps, core_ids=list(range(N_CORES)))
    return np.stack([res.results[b]["y"] for b in range(B)], axis=0).astype(np.float32)


# revision 7
# speedup vs baseline: 8.7269x; 8.7269x over previous
"""Trainium2 Bass kernel for BitFlipLinear: y[b,s,o] = sum_i x[b,s,i]*W[o,i] + bias[o].

Strategy
--------
Data-parallel over the batch dim: each of the 8 NeuronCores computes one
[4096,4096] @ [4096,4096]^T matmul (137 GFLOP/core).

Precision trick: W's values are {0,1,3} - exactly representable in bf16.
Split x = x_hi + x_lo (x_hi = bf16(x), x_lo = bf16(x - x_hi)) and run TWO
bf16 matmuls accumulated in fp32 PSUM.  Residual error ~2^-18 per element,
i.e. effectively fp32-accurate output at bf16 TensorE speed.

Per-core pipeline (single NEFF, Tile-scheduled):
  phase W : cast W fp32 -> bf16 in DRAM (SWDGE cast-DMA, DRAM->DRAM)
  phase X : per 128-row block, load x fp32, DVE-split into x_hi/x_lo bf16,
            write back to DRAM scratch
  matmul  : for each o-half (2048 cols): keep W^T[:, o_half] bf16 resident in
            SBUF (16.75 MB, loaded once via XBAR transpose-DMA); stream
            x_hi^T/x_lo^T 128-row tiles via transpose-DMA; accumulate
            4 PSUM banks per s-tile over all 32 k-tiles x {hi,lo}.
            The bias is pre-loaded into PSUM with a K=2 matmul
            (ones[2,128]^T @ [bias_hi;bias_lo][2,512]) as the start=True
            instruction of each accumulation group, so the epilogue is a
            plain PSUM->SBUF copy + DMA out.
"""

import os
import sys

for _p in ("/opt/trn_rl_repo",):
    if os.path.isdir(_p) and _p not in sys.path:
        sys.path.append(_p)

import numpy as np

B, S, K, O = 8, 4096, 4096, 4096
N_CORES = 8
O_CHUNK = 2048          # o columns resident per pass (4 PSUM banks of 512 fp32)
CONV_I = 512            # i-chunk for the x hi/lo split stage
W_CAST_ROWS = 1024      # rows per W fp32->bf16 cast DMA
BIAS_CH = 1024          # bias prep chunk

_NC_CACHE = {}


def build_nc(S=S, K=K, O=O, O_CHUNK=O_CHUNK, enable_asserts=False, repeat=1):
    """Build + compile the single-core Bass program (same NEFF runs on all cores).

    repeat>1 re-emits the whole pipeline N times (timing runs only: lets the
    per-dispatch overhead be cancelled out via a time delta between repeats).
    """
    import concourse.bacc as bacc
    import concourse.tile as tile
    import concourse.mybir as mybir

    f32 = mybir.dt.float32
    bf16 = mybir.dt.bfloat16
    P = 128
    NB = O_CHUNK // 512       # PSUM banks per o-chunk
    NOH = O // O_CHUNK        # number of o-chunks
    ST = S // P               # s-tiles
    KO = K // P               # k-tiles (contraction)
    NCV = K // CONV_I         # conv chunks per s-tile

    assert S % P == 0 and K % P == 0 and O % O_CHUNK == 0 and O_CHUNK % 512 == 0

    nc = bacc.Bacc("TRN2", target_bir_lowering=False, debug=False,
                   enable_asserts=enable_asserts)

    ap_x = nc.dram_tensor("x", [S, K], f32, kind="ExternalInput").ap()
    ap_w = nc.dram_tensor("w", [O, K], f32, kind="ExternalInput").ap()
    ap_bias = nc.dram_tensor("bias", [O], f32, kind="ExternalInput").ap()
    ap_y = nc.dram_tensor("y", [S, O], f32, kind="ExternalOutput").ap()

    with tile.TileContext(nc) as tc:
        with (
            tc.tile_pool(name="dram", bufs=1, space="DRAM") as dram,
            tc.tile_pool(name="const", bufs=1) as const,
            tc.tile_pool(name="bstage", bufs=1) as bstage,
            tc.tile_pool(name="convin", bufs=2) as convin,
            tc.tile_pool(name="convout", bufs=2) as convout,
            tc.tile_pool(name="wres", bufs=1) as wresp,
            tc.tile_pool(name="xts", bufs=2) as xtsp,
            tc.tile_pool(name="outp", bufs=2) as outp,
            tc.tile_pool(name="psum", bufs=2, space="PSUM") as psum,
        ):
            wb = dram.tile([O, K], bf16)     # bf16 copy of W
            xhi = dram.tile([S, K], bf16)    # bf16 high part of x
            xlo = dram.tile([S, K], bf16)    # bf16 low (residual) part of x

            # whole pipeline; repeated `repeat` times (timing builds only)
            for _rep in range(repeat):
              # bias prep: bias2[0,:]=bf16(bias), bias2[1,:]=bf16(bias-hi)
              bias2 = const.tile([2, O], bf16)
              ones2 = const.tile([2, P], bf16)
              nc.vector.memset(ones2[:], 1.0)
              bch = min(BIAS_CH, O)
              for c in range(O // bch):
                sl = slice(c * bch, (c + 1) * bch)
                bst = bstage.tile([1, bch], f32)
                nc.scalar.dma_start(bst[:], ap_bias[None, sl])
                nc.vector.tensor_copy(bias2[0:1, sl], bst[:])
                # DVE can't write at base partition 1 -> stage lo on partition
                # 0 and move it with a small SBUF->SBUF DMA.
                blo = bstage.tile([1, bch], bf16, tag="blo")
                nc.vector.tensor_sub(blo[:], bst[:], bias2[0:1, sl])
                nc.scalar.dma_start(bias2[1:2, sl], blo[:])

              # W fp32 -> bf16 (DRAM->DRAM cast on SWDGE)
              wch = min(W_CAST_ROWS, O)
              for c in range(O // wch):
                sl = slice(c * wch, (c + 1) * wch)
                nc.gpsimd.dma_start(wb[sl, :], ap_w[sl, :])

              for oh in range(NOH):
                # resident W^T slab: wres[pi, ko, o] = W[oh*O_CHUNK+o, ko*128+pi]
                wres = wresp.tile([P, KO, O_CHUNK], bf16)
                for c in range(O_CHUNK // 512):
                    o0 = oh * O_CHUNK + c * 512
                    nc.sync.dma_start(
                        wres[:, :, c * 512:(c + 1) * 512],
                        wb[o0:o0 + 512, :],
                        transpose=True,
                    )

                for st in range(ST):
                    rows = slice(st * P, (st + 1) * P)
                    if oh == 0:
                        # split x rows into hi/lo bf16, store to DRAM scratch
                        for c in range(NCV):
                            cols = slice(c * CONV_I, (c + 1) * CONV_I)
                            xin = convin.tile([P, CONV_I], f32)
                            nc.scalar.dma_start(xin[:], ap_x[rows, cols])
                            ch = convout.tile([P, 2, CONV_I], bf16)
                            nc.vector.tensor_copy(ch[:, 0], xin[:])
                            nc.vector.tensor_sub(ch[:, 1], xin[:], ch[:, 0])
                            nc.scalar.dma_start(xhi[rows, cols], ch[:, 0])
                            nc.scalar.dma_start(xlo[rows, cols], ch[:, 1])

                    # transposed x tiles: xt[pi, h, ko, s] = x_h[s, ko*128+pi]
                    xt = xtsp.tile([P, 2, KO, P], bf16)
                    nc.sync.dma_start(xt[:, 0], xhi[rows, :], transpose=True)
                    nc.sync.dma_start(xt[:, 1], xlo[rows, :], transpose=True)

                    pt = psum.tile([P, O_CHUNK], f32)
                    # bias seeds each accumulation group (start=True clears bank)
                    for ob in range(NB):
                        b0 = oh * O_CHUNK + ob * 512
                        nc.tensor.matmul(
                            pt[:, ob * 512:(ob + 1) * 512],
                            ones2[:], bias2[:, b0:b0 + 512],
                            start=True, stop=False,
                        )
                    for ko in range(KO):
                        for h in range(2):
                            last = (ko == KO - 1) and (h == 1)
                            lhsT = xt[:, h, ko, :]
                            for ob in range(NB):
                                nc.tensor.matmul(
                                    pt[:, ob * 512:(ob + 1) * 512],
                                    lhsT,
                                    wres[:, ko, ob * 512:(ob + 1) * 512],
                                    start=False, stop=last,
                                )

                    ot = outp.tile([P, O_CHUNK], f32)
                    nc.any.tensor_copy(ot[:], pt[:])
                    nc.scalar.dma_start(
                        ap_y[rows, oh * O_CHUNK:(oh + 1) * O_CHUNK], ot[:]
                    )
              # end oh loop (per repeat)

    nc.compile()
    return nc


def _get_nc():
    key = (S, K, O, O_CHUNK)
    if key not in _NC_CACHE:
        _NC_CACHE[key] = build_nc(S, K, O, O_CHUNK)
    return _NC_CACHE[key]


def make_in_maps(x, weight, bias):
    x = np.ascontiguousarray(np.asarray(x, dtype=np.float32))
    weight = np.ascontiguousarray(np.asarray(weight, dtype=np.float32))
    bias = np.ascontiguousarray(np.asarray(bias, dtype=np.float32))
    assert x.shape == (B, S, K), x.shape
    return [
        {"x": np.ascontiguousarray(x[b]), "w": weight, "bias": bias}
        for b in range(B)
    ]


def kernel(x, weight, bias):
    from concourse.bass_utils import run_bass_kernel_spmd

    nc = _get_nc()
    in_maps = make_in_maps(x, weight, bias)
    res = run_bass_kernel_spmd(nc, in_maps, core_ids=list(range(N_CORES)))
    return np.stack([res.results[b]["y"] for b in range(B)], axis=0).astype(np.float32)
